# revision 46
# baseline (speedup 1.0000x reference)
"""Trainium2 Bass kernel for the clustered spatial-consistency (SC2-PCR) loss.

Problem: 64 contiguous clusters of 512 points each (N=32768, 3-D). Per
cluster compute the 512x512 pairwise-distance matrices of src (pc1) and
tgt (pc1+flow); loss = mean(min((d_s-d_t)^2, th^2)/th^2) over all pairs
and clusters. Sharded 8 clusters per core across 8 NeuronCores.

Math (division form instead of two sqrts):
    v = d_s - d_t = (sq_s - sq_t) / (d_s + d_t)
    (d_s + d_t)^2 ~= 4*(sq_s + eps)   [self-limiting error: the approx
        error is O(v/d) relative, and only |v|<=th pairs matter, where
        v/d <= th/d_min ~ 0.4%]
    w = v/th = delta * r,  delta = sq_s - sq_t  (PE, K=42 matmul)
    r = AbsRsqrt(4*th^2 * (sq_s + eps))         (ACT, one table, exact
        to 4e-5; Rsqrt/Reciprocal are banned but Abs_reciprocal_sqrt
        is accurate)
    sq_s + eps comes from a second cheap matmul  (PE, K=13)
    loss elem = min(w^2, 1)

Per 128-row block only columns >= block start are computed (symmetry):
full sum = 2*strip_sums - diag_block_sums (b=3 strips ARE diag blocks,
so the host reuses their strip sums as diag sums).

Engines per unit (n_cl clusters x one row block; separate sigma/delta
PSUM pools so sigma tiles free right after AbsRsqrt and the PE streams):
    PE:   2*n_cl matmuls (bf16: K=13 sigma, K=42 delta), W=512-128b cols
    ACT:  r = AbsRsqrt(S*scale) PSUM->bf16; Square+accum_out (strip sums)
    DVE:  w = delta*r (PSUM fp32 x bf16, 1x); clamp (bf16 4x);
          diag re-accumulate from scr = wc^2 (tensor_scalar add+accum)
Measured notes: DVE accum_out ops run at ~1x; non-accum bf16
tensor_scalar hits 4x; GpSimd tensor ops are ~0.42-0.73 efficiency and
PSUM-blind, so it only idles; Reciprocal/Rsqrt are banned but
Abs_reciprocal_sqrt measures 4e-5 max rel err on HW.

Operand layout (host-packed bf16, shared moving operand R):
  rows 0-8:  s-products (h,m,h)/coord; Ld=-2(h,h,m), Ls=-2(h,h,m)
  rows 9-10: R=1;  Ls=split2(ns+eps/2), Ld=split2(ns-nt)
  rows 11-12: R=split2(ns+eps/2) j-side; Ls=1, Ld=0
  -> sigma matmul is rows 0:13 (contiguous K=13)
  rows 13-14: R=split2(ns-nt) j-side; Ld=1
  rows 15-23: s-products (m,l,h)/coord; Ld=-2(m,h,l)
  rows 24-41: t-products 6/coord; Ld=+2
  -> delta matmul is rows 0:42
"""

import numpy as np
import ml_dtypes

N_POINTS = 32768
NUM_CLUSTERS = 64
M = N_POINTS // NUM_CLUSTERS          # 512 points per cluster
N_CORES = 8
CLUSTERS_PER_CORE = NUM_CLUSTERS // N_CORES   # 8
PTS_PER_CORE = CLUSTERS_PER_CORE * M  # 4096
D_THRE = 0.03
TH2 = D_THRE * D_THRE
EPS = 0.25
K_DELTA = 42
K_SIGMA = 13
N_BLOCKS = M // 128                   # 4 row blocks per cluster

# units: (n_clusters, cluster-group index, row-block). Each unit fills one
# [128,1024] sigma PSUM tile and one [128,1024] delta tile (2 banks each).
UNITS = (
    [(2, 0, 0), (2, 1, 0), (2, 0, 1), (2, 1, 1), (4, 0, 2)]
    + [(2, 2, 0), (2, 3, 0), (2, 2, 1), (2, 3, 1), (4, 1, 2)]
    + [(8, 0, 3)]
)
N_UNITS = len(UNITS)
B3_UNITS = [u for u, (_, _, b) in enumerate(UNITS) if b == 3]

_COMPILED = {}


def _split3(x):
    x = x.astype(np.float32)
    h = x.astype(ml_dtypes.bfloat16)
    r = x - h.astype(np.float32)
    m = r.astype(ml_dtypes.bfloat16)
    l = (r - m.astype(np.float32)).astype(ml_dtypes.bfloat16)
    return h, m, l


def _split2(x):
    x = x.astype(np.float32)
    h = x.astype(ml_dtypes.bfloat16)
    l = (x - h.astype(np.float32)).astype(ml_dtypes.bfloat16)
    return h, l


def _build_operands(P, T):
    """P, T: [4096, 3] fp32 src/tgt points -> R[42,n], Ld[42,n], Ls[13,n]."""
    bf16 = ml_dtypes.bfloat16
    n = P.shape[0]
    R = np.zeros((K_DELTA, n), dtype=bf16)
    Ld = np.zeros((K_DELTA, n), dtype=bf16)
    Ls = np.zeros((K_SIGMA, n), dtype=bf16)
    hs, ms, ls = [], [], []
    ht, mt, lt = [], [], []
    for c in range(3):
        a, b, d = _split3(P[:, c])
        hs.append(a); ms.append(b); ls.append(d)
        a, b, d = _split3(T[:, c])
        ht.append(a); mt.append(b); lt.append(d)

    def neg2(x):
        return (-2.0 * x.astype(np.float32)).astype(bf16)

    def pos2(x):
        return (2.0 * x.astype(np.float32)).astype(bf16)

    # rows 0-8: s products hh, hm, mh
    for c in range(3):
        R[3 * c + 0] = hs[c]; Ld[3 * c + 0] = neg2(hs[c])
        R[3 * c + 1] = ms[c]; Ld[3 * c + 1] = neg2(hs[c])
        R[3 * c + 2] = hs[c]; Ld[3 * c + 2] = neg2(ms[c])
    Ls[0:9] = Ld[0:9]

    ns = np.einsum("nc,nc->n", P.astype(np.float64), P.astype(np.float64))
    nt = np.einsum("nc,nc->n", T.astype(np.float64), T.astype(np.float64))
    sn_h, sn_l = _split2((ns + EPS / 2).astype(np.float32))
    dn_h, dn_l = _split2((ns - nt).astype(np.float32))
    one = np.ones(n, dtype=bf16)
    # rows 9-10: i-side norms (R=1)
    R[9] = one; Ls[9] = sn_h; Ld[9] = dn_h
    R[10] = one; Ls[10] = sn_l; Ld[10] = dn_l
    # rows 11-12: sigma j-side norms
    R[11] = sn_h; Ls[11] = one
    R[12] = sn_l; Ls[12] = one
    # rows 13-14: delta j-side norms
    R[13] = dn_h; Ld[13] = one
    R[14] = dn_l; Ld[14] = one
    # rows 15-23: s products mm, hl, lh
    for c in range(3):
        R[15 + 3 * c + 0] = ms[c]; Ld[15 + 3 * c + 0] = neg2(ms[c])
        R[15 + 3 * c + 1] = ls[c]; Ld[15 + 3 * c + 1] = neg2(hs[c])
        R[15 + 3 * c + 2] = hs[c]; Ld[15 + 3 * c + 2] = neg2(ls[c])
    # rows 24-41: t products hh, hm, mh, mm, hl, lh (+2)
    for c in range(3):
        base = 24 + 6 * c
        R[base + 0] = ht[c]; Ld[base + 0] = pos2(ht[c])
        R[base + 1] = mt[c]; Ld[base + 1] = pos2(ht[c])
        R[base + 2] = ht[c]; Ld[base + 2] = pos2(mt[c])
        R[base + 3] = mt[c]; Ld[base + 3] = pos2(mt[c])
        R[base + 4] = lt[c]; Ld[base + 4] = pos2(ht[c])
        R[base + 5] = ht[c]; Ld[base + 5] = pos2(lt[c])
    return R, Ld, Ls


def _build_bass(loop_n=0):
    import contextlib
    import concourse.bacc as bacc
    import concourse.mybir as mybir
    import concourse.tile as tile

    f32 = mybir.dt.float32
    bf16 = mybir.dt.bfloat16
    Alu = mybir.AluOpType
    Act = mybir.ActivationFunctionType

    nc = bacc.Bacc("TRN2", target_bir_lowering=False, debug=False)

    d_R = nc.dram_tensor("R", [K_DELTA, PTS_PER_CORE], bf16, kind="ExternalInput")
    d_Ld = nc.dram_tensor("Ld", [K_DELTA, PTS_PER_CORE], bf16, kind="ExternalInput")
    d_Ls = nc.dram_tensor("Ls", [K_SIGMA, PTS_PER_CORE], bf16, kind="ExternalInput")
    d_out = nc.dram_tensor("out", [128, 2 * N_UNITS], f32, kind="ExternalOutput")

    RSCALE = 4.0 * TH2  # r = 1/sqrt(RSCALE*(sq_s+eps)) = 1/(2 th sqrt(sq+eps))

    with tile.TileContext(nc) as tc:
        with (
            tc.tile_pool(name="ops", bufs=1) as ops_pool,
            tc.tile_pool(name="psA", bufs=2, space="PSUM") as psA_pool,
            tc.tile_pool(name="psB", bufs=2, space="PSUM") as psB_pool,
            tc.tile_pool(name="work", bufs=6) as work_pool,
            tc.tile_pool(name="accp", bufs=1) as acc_pool,
        ):
            sR = ops_pool.tile([K_DELTA, PTS_PER_CORE], bf16, tag="sR")
            sLs = ops_pool.tile([K_SIGMA, PTS_PER_CORE], bf16, tag="sLs")
            sLd = ops_pool.tile([K_DELTA, PTS_PER_CORE], bf16, tag="sLd")
            # Ls (13 rows) first, then R: primes the sigma->AbsRsqrt path
            # ~2us before Ld (delta operand) lands
            nc.sync.dma_start(out=sLs[:], in_=d_Ls[:])
            nc.sync.dma_start(out=sR[:], in_=d_R[:])
            nc.sync.dma_start(out=sLd[:], in_=d_Ld[:])

            # acc[:, u] = strip sums; acc[:, N_UNITS+u] = diag sums
            acc = acc_pool.tile([128, 2 * N_UNITS], f32, tag="acc")
            nc.vector.memset(acc[:], 0.0)

            # force the ACT table load now, during the input-DMA wait
            dummy = acc_pool.tile([128, 1], f32, tag="dummy")
            nc.vector.memset(dummy[:], 1.0)
            nc.scalar.activation(dummy[:], dummy[:], Act.Abs_reciprocal_sqrt)

            def emit_head(u):
                """matmuls + AbsRsqrt + mult + clamp for unit u; returns wc."""
                n_cl, idx, b = UNITS[u]
                W = M - b * 128
                stride = 1024 // n_cl   # psum offset per cluster (>= W)
                clusters = [idx * n_cl + k for k in range(n_cl)]

                # separate sigma/delta PSUM tiles: sigma frees right after
                # AbsRsqrt, so the PE can run ahead into the next unit
                psS = psA_pool.tile([128, 1024], f32, tag="psS", name="psS")
                psD = psB_pool.tile([128, 1024], f32, tag="psD", name="psD")
                for k, cc in enumerate(clusters):
                    lo, hi = cc * M + b * 128, (cc + 1) * M
                    nc.tensor.matmul(
                        psS[:, k * stride:k * stride + W],
                        sLs[0:K_SIGMA, lo:lo + 128],
                        sR[0:K_SIGMA, lo:hi],
                        start=True, stop=True,
                    )
                for k, cc in enumerate(clusters):
                    lo, hi = cc * M + b * 128, (cc + 1) * M
                    nc.tensor.matmul(
                        psD[:, k * stride:k * stride + W],
                        sLd[0:K_DELTA, lo:lo + 128],
                        sR[0:K_DELTA, lo:hi],
                        start=True, stop=True,
                    )

                S_v = psS[:].rearrange("p (c w) -> p c w", c=n_cl)[:, :, 0:W]
                D_v = psD[:].rearrange("p (c w) -> p c w", c=n_cl)[:, :, 0:W]

                # r = 1/(2 th sqrt(sq_s+eps))  [ACT]
                r = work_pool.tile([128, n_cl * W], bf16, tag="r", name="r")
                r_v = r[:].rearrange("p (c w) -> p c w", c=n_cl)
                nc.scalar.activation(
                    r_v, S_v, Act.Abs_reciprocal_sqrt, scale=RSCALE
                )

                # w = delta * r  [DVE, PSUM fp32 x bf16 -> bf16]
                w = work_pool.tile([128, n_cl * W], bf16, tag="w", name="w")
                w_v = w[:].rearrange("p (c w) -> p c w", c=n_cl)
                nc.vector.tensor_tensor(w_v, D_v, r_v, Alu.mult)

                # wc = clamp(w, [-1,1]) on GpSimd: frees DVE, and the
                # clamp->sqacc boundary is a cross-engine hop either way
                wc = work_pool.tile([128, n_cl * W], bf16, tag="wc", name="wc")
                nc.gpsimd.tensor_scalar(
                    wc[:], w[:], 1.0, -1.0, Alu.min, Alu.max
                )
                return wc

            def emit_tail(u, wc):
                """square+accum (ACT) and diag re-accum (DVE) for unit u."""
                n_cl, idx, b = UNITS[u]
                W = M - b * 128
                # acc[u] = sum(wc^2); scr = wc^2 feeds the diag re-sum;
                # host computes full = 2*acc - acc_diag.
                scr = work_pool.tile([128, n_cl * W], bf16, tag="scr", name="scr")
                if b == 3:  # last tail on DVE so ACT's span ends earlier
                    nc.vector.scalar_tensor_tensor(
                        scr[:], wc[:], 1.0, wc[:], Alu.mult, Alu.mult,
                        accum_out=acc[:, u:u + 1],
                    )
                else:
                    nc.scalar.activation(
                        scr[:], wc[:], Act.Square, accum_out=acc[:, u:u + 1],
                    )
                if b < 3:  # b3 strips ARE diag blocks; host reuses acc[u]
                    scr_v = scr[:].rearrange("p (c w) -> p c w", c=n_cl)
                    scrD = work_pool.tile(
                        [128, n_cl * 128], bf16, tag="scrD", name="scrD"
                    )
                    scrD_v = scrD[:].rearrange("p (c w) -> p c w", c=n_cl)
                    nc.vector.tensor_scalar(
                        scrD_v, scr_v[:, :, 0:128], 0.0, None, Alu.add, Alu.add,
                        accum_out=acc[:, N_UNITS + u:N_UNITS + u + 1],
                    )

            loop_cm = tc.For_i(0, loop_n, 1) if loop_n else contextlib.nullcontext()
            with loop_cm:
              # software-pipelined emission with a 1-unit lag: unit u+1's
              # AbsRsqrt sits ahead of unit u's Square in the ACT FIFO, so
              # the sigma path is never blocked behind the delta chain.
              prev = None
              for u in range(N_UNITS):
                wc = emit_head(u)
                if prev is not None:
                    emit_tail(u - 1, prev)
                prev = wc
              emit_tail(N_UNITS - 1, prev)

            # DMA the per-unit accumulators out; host does the final sum.
            # Issued from the ACT queue, which is idle by then.
            nc.scalar.dma_start(out=d_out[:], in_=acc[:])

    nc.compile()
    return nc


def _get_compiled(loop_n=0):
    key = loop_n
    if key not in _COMPILED:
        _COMPILED[key] = _build_bass(loop_n=loop_n)
    return _COMPILED[key]


def _make_in_maps(pc, tg):
    in_maps = []
    for c in range(N_CORES):
        sl = slice(c * PTS_PER_CORE, (c + 1) * PTS_PER_CORE)
        R, Ld, Ls = _build_operands(pc[sl], tg[sl])
        in_maps.append({"R": R, "Ld": Ld, "Ls": Ls})
    return in_maps


def kernel(flow, pc1, labels, num_clusters):
    from concourse.bass_utils import run_bass_kernel_spmd

    pc = np.ascontiguousarray(np.asarray(pc1, dtype=np.float32)[0])    # [N,3]
    fl = np.ascontiguousarray(np.asarray(flow, dtype=np.float32)[0])   # [N,3]
    tg = (pc + fl).astype(np.float32)

    in_maps = _make_in_maps(pc, tg)
    nc = _get_compiled()
    res = run_bass_kernel_spmd(nc, in_maps, core_ids=list(range(N_CORES)))
    total = 0.0
    for r in res.results:
        total += 2.0 * float(r["out"][:, :N_UNITS].sum(dtype=np.float64))
        total -= float(r["out"][:, N_UNITS:].sum(dtype=np.float64))
        for u in B3_UNITS:  # b3 strips are pure diag blocks
            total -= float(r["out"][:, u].sum(dtype=np.float64))
    loss = total / (M * M * NUM_CLUSTERS)
    return np.float32(loss)


def _numpy_check():
    """Validate the math (not the PE rounding) against the reference formula."""
    rng = np.random.default_rng(0)
    P = (rng.standard_normal((1024, 3)) * 20.0).astype(np.float32)
    F = (rng.standard_normal((1024, 3)) * 0.1).astype(np.float32)
    T = P + F
    # reference per 512-cluster
    tot_ref = 0.0
    tot_new = 0.0
    for c in range(2):
        p = P[c * 512:(c + 1) * 512].astype(np.float64)
        t = T[c * 512:(c + 1) * 512].astype(np.float64)
        ds = np.sqrt(((p[:, None] - p[None]) ** 2).sum(-1))
        dt = np.sqrt(((t[:, None] - t[None]) ** 2).sum(-1))
        tot_ref += np.minimum((ds - dt) ** 2, TH2).sum() / TH2
        sqs = ((p[:, None] - p[None]) ** 2).sum(-1)
        sqt = ((t[:, None] - t[None]) ** 2).sum(-1)
        delta = sqs - sqt
        r = 1.0 / np.sqrt(4 * TH2 * (sqs + EPS))
        w = np.clip(delta * r, -1, 1)
        tot_new += (w ** 2).sum()
    print("numpy rel err:", abs(tot_new - tot_ref) / tot_ref)


if __name__ == "__main__":
    _numpy_check()


# revision 47
# speedup vs baseline: 1.1683x; 1.1683x over previous
"""Trainium2 Bass kernel for the clustered spatial-consistency (SC2-PCR) loss.

Problem: 64 contiguous clusters of 512 points each (N=32768, 3-D). Per
cluster compute the 512x512 pairwise-distance matrices of src (pc1) and
tgt (pc1+flow); loss = mean(min((d_s-d_t)^2, th^2)/th^2) over all pairs
and clusters. Sharded 8 clusters per core across 8 NeuronCores.

Math (division form instead of two sqrts):
    v = d_s - d_t = (sq_s - sq_t) / (d_s + d_t)
    (d_s + d_t)^2 ~= 4*(sq_s + eps)   [self-limiting error: the approx
        error is O(v/d) relative, and only |v|<=th pairs matter, where
        v/d <= th/d_min ~ 0.4%]
    w = v/th = delta * r,  delta = sq_s - sq_t  (PE, K=42 matmul)
    r = AbsRsqrt(4*th^2 * (sq_s + eps))         (ACT, one table, exact
        to 4e-5; Rsqrt/Reciprocal are banned but Abs_reciprocal_sqrt
        is accurate)
    sq_s + eps comes from a second cheap matmul  (PE, K=13)
    loss elem = min(w^2, 1)

Per 128-row block only columns >= block start are computed (symmetry):
full sum = 2*strip_sums - diag_block_sums (b=3 strips ARE diag blocks,
so the host reuses their strip sums as diag sums).

Engines per unit (n_cl clusters x one row block; separate sigma/delta
PSUM pools so sigma tiles free right after AbsRsqrt and the PE streams):
    PE:   2*n_cl matmuls (bf16: K=13 sigma, K=42 delta), W=512-128b cols
    ACT:  r = AbsRsqrt(S*scale) PSUM->bf16; Square+accum_out (strip sums)
    DVE:  w = delta*r (PSUM fp32 x bf16, 1x); clamp (bf16 4x);
          diag re-accumulate from scr = wc^2 (tensor_scalar add+accum)
Measured notes: DVE accum_out ops run at ~1x; non-accum bf16
tensor_scalar hits 4x; GpSimd tensor ops are ~0.42-0.73 efficiency and
PSUM-blind, so it only idles; Reciprocal/Rsqrt are banned but
Abs_reciprocal_sqrt measures 4e-5 max rel err on HW.

Operand layout (host-packed bf16, shared moving operand R):
  rows 0-8:  s-products (h,m,h)/coord; Ld=-2(h,h,m), Ls=-2(h,h,m)
  rows 9-10: R=1;  Ls=split2(ns+eps/2), Ld=split2(ns-nt)
  rows 11-12: R=split2(ns+eps/2) j-side; Ls=1, Ld=0
  -> sigma matmul is rows 0:13 (contiguous K=13)
  rows 13-14: R=split2(ns-nt) j-side; Ld=1
  rows 15-23: s-products (m,l,h)/coord; Ld=-2(m,h,l)
  rows 24-41: t-products 6/coord; Ld=+2
  -> delta matmul is rows 0:42
"""

import numpy as np
import ml_dtypes

N_POINTS = 32768
NUM_CLUSTERS = 64
M = N_POINTS // NUM_CLUSTERS          # 512 points per cluster
N_CORES = 8
CLUSTERS_PER_CORE = NUM_CLUSTERS // N_CORES   # 8
PTS_PER_CORE = CLUSTERS_PER_CORE * M  # 4096
D_THRE = 0.03
TH2 = D_THRE * D_THRE
EPS = 0.25
K_DELTA = 42
K_SIGMA = 13
N_BLOCKS = M // 128                   # 4 row blocks per cluster

# units: (n_clusters, cluster-group index, row-block). Each unit fills one
# [128,1024] sigma PSUM tile and one [128,1024] delta tile (2 banks each).
UNITS = (
    [(2, 0, 0), (2, 1, 0), (2, 0, 1), (2, 1, 1), (4, 0, 2)]
    + [(2, 2, 0), (2, 3, 0), (2, 2, 1), (2, 3, 1), (4, 1, 2)]
    + [(8, 0, 3)]
)
N_UNITS = len(UNITS)
B3_UNITS = [u for u, (_, _, b) in enumerate(UNITS) if b == 3]

_COMPILED = {}


def _split3(x):
    x = x.astype(np.float32)
    h = x.astype(ml_dtypes.bfloat16)
    r = x - h.astype(np.float32)
    m = r.astype(ml_dtypes.bfloat16)
    l = (r - m.astype(np.float32)).astype(ml_dtypes.bfloat16)
    return h, m, l


def _split2(x):
    x = x.astype(np.float32)
    h = x.astype(ml_dtypes.bfloat16)
    l = (x - h.astype(np.float32)).astype(ml_dtypes.bfloat16)
    return h, l


def _build_operands(P, T):
    """P, T: [4096, 3] fp32 src/tgt points -> R[42,n], Ld[42,n], Ls[13,n]."""
    bf16 = ml_dtypes.bfloat16
    n = P.shape[0]
    R = np.zeros((K_DELTA, n), dtype=bf16)
    Ld = np.zeros((K_DELTA, n), dtype=bf16)
    Ls = np.zeros((K_SIGMA, n), dtype=bf16)
    hs, ms, ls = [], [], []
    ht, mt, lt = [], [], []
    for c in range(3):
        a, b, d = _split3(P[:, c])
        hs.append(a); ms.append(b); ls.append(d)
        a, b, d = _split3(T[:, c])
        ht.append(a); mt.append(b); lt.append(d)

    def neg2(x):
        return (-2.0 * x.astype(np.float32)).astype(bf16)

    def pos2(x):
        return (2.0 * x.astype(np.float32)).astype(bf16)

    # rows 0-8: s products hh, hm, mh
    for c in range(3):
        R[3 * c + 0] = hs[c]; Ld[3 * c + 0] = neg2(hs[c])
        R[3 * c + 1] = ms[c]; Ld[3 * c + 1] = neg2(hs[c])
        R[3 * c + 2] = hs[c]; Ld[3 * c + 2] = neg2(ms[c])
    Ls[0:9] = Ld[0:9]

    ns = np.einsum("nc,nc->n", P.astype(np.float64), P.astype(np.float64))
    nt = np.einsum("nc,nc->n", T.astype(np.float64), T.astype(np.float64))
    sn_h, sn_l = _split2((ns + EPS / 2).astype(np.float32))
    dn_h, dn_l = _split2((ns - nt).astype(np.float32))
    one = np.ones(n, dtype=bf16)
    # rows 9-10: i-side norms (R=1)
    R[9] = one; Ls[9] = sn_h; Ld[9] = dn_h
    R[10] = one; Ls[10] = sn_l; Ld[10] = dn_l
    # rows 11-12: sigma j-side norms
    R[11] = sn_h; Ls[11] = one
    R[12] = sn_l; Ls[12] = one
    # rows 13-14: delta j-side norms
    R[13] = dn_h; Ld[13] = one
    R[14] = dn_l; Ld[14] = one
    # rows 15-23: s products mm, hl, lh
    for c in range(3):
        R[15 + 3 * c + 0] = ms[c]; Ld[15 + 3 * c + 0] = neg2(ms[c])
        R[15 + 3 * c + 1] = ls[c]; Ld[15 + 3 * c + 1] = neg2(hs[c])
        R[15 + 3 * c + 2] = hs[c]; Ld[15 + 3 * c + 2] = neg2(ls[c])
    # rows 24-41: t products hh, hm, mh, mm, hl, lh (+2)
    for c in range(3):
        base = 24 + 6 * c
        R[base + 0] = ht[c]; Ld[base + 0] = pos2(ht[c])
        R[base + 1] = mt[c]; Ld[base + 1] = pos2(ht[c])
        R[base + 2] = ht[c]; Ld[base + 2] = pos2(mt[c])
        R[base + 3] = mt[c]; Ld[base + 3] = pos2(mt[c])
        R[base + 4] = lt[c]; Ld[base + 4] = pos2(ht[c])
        R[base + 5] = ht[c]; Ld[base + 5] = pos2(lt[c])
    return R, Ld, Ls


def _build_bass(loop_n=0):
    import contextlib
    import concourse.bacc as bacc
    import concourse.mybir as mybir
    import concourse.tile as tile

    f32 = mybir.dt.float32
    bf16 = mybir.dt.bfloat16
    Alu = mybir.AluOpType
    Act = mybir.ActivationFunctionType

    nc = bacc.Bacc("TRN2", target_bir_lowering=False, debug=False)

    d_R = nc.dram_tensor("R", [K_DELTA, PTS_PER_CORE], bf16, kind="ExternalInput")
    d_Ld = nc.dram_tensor("Ld", [K_DELTA, PTS_PER_CORE], bf16, kind="ExternalInput")
    d_Ls = nc.dram_tensor("Ls", [K_SIGMA, PTS_PER_CORE], bf16, kind="ExternalInput")
    d_out = nc.dram_tensor("out", [128, 2 * N_UNITS], f32, kind="ExternalOutput")

    RSCALE = 4.0 * TH2  # r = 1/sqrt(RSCALE*(sq_s+eps)) = 1/(2 th sqrt(sq+eps))

    with tile.TileContext(nc) as tc:
        with (
            tc.tile_pool(name="ops", bufs=1) as ops_pool,
            tc.tile_pool(name="psA", bufs=2, space="PSUM") as psA_pool,
            tc.tile_pool(name="psB", bufs=2, space="PSUM") as psB_pool,
            tc.tile_pool(name="work", bufs=6) as work_pool,
            tc.tile_pool(name="accp", bufs=1) as acc_pool,
        ):
            sR = ops_pool.tile([K_DELTA, PTS_PER_CORE], bf16, tag="sR")
            sLs = ops_pool.tile([K_SIGMA, PTS_PER_CORE], bf16, tag="sLs")
            sLd = ops_pool.tile([K_DELTA, PTS_PER_CORE], bf16, tag="sLd")
            # Ls (13 rows) first, then R: primes the sigma->AbsRsqrt path
            # ~2us before Ld (delta operand) lands
            nc.sync.dma_start(out=sLs[:], in_=d_Ls[:])
            nc.sync.dma_start(out=sR[:], in_=d_R[:])
            nc.sync.dma_start(out=sLd[:], in_=d_Ld[:])

            # acc[:, u] = strip sums; acc[:, N_UNITS+u] = diag sums
            acc = acc_pool.tile([128, 2 * N_UNITS], f32, tag="acc")
            nc.vector.memset(acc[:], 0.0)

            # force the ACT table load now, during the input-DMA wait
            dummy = acc_pool.tile([128, 1], f32, tag="dummy")
            nc.vector.memset(dummy[:], 1.0)
            nc.scalar.activation(dummy[:], dummy[:], Act.Abs_reciprocal_sqrt)

            def emit_head(u):
                """matmuls + AbsRsqrt + mult + clamp for unit u; returns wc."""
                n_cl, idx, b = UNITS[u]
                W = M - b * 128
                stride = 1024 // n_cl   # psum offset per cluster (>= W)
                clusters = [idx * n_cl + k for k in range(n_cl)]

                # separate sigma/delta PSUM tiles: sigma frees right after
                # AbsRsqrt, so the PE can run ahead into the next unit
                psS = psA_pool.tile([128, 1024], f32, tag="psS", name="psS")
                psD = psB_pool.tile([128, 1024], f32, tag="psD", name="psD")
                for k, cc in enumerate(clusters):
                    lo, hi = cc * M + b * 128, (cc + 1) * M
                    nc.tensor.matmul(
                        psS[:, k * stride:k * stride + W],
                        sLs[0:K_SIGMA, lo:lo + 128],
                        sR[0:K_SIGMA, lo:hi],
                        start=True, stop=True,
                    )
                for k, cc in enumerate(clusters):
                    lo, hi = cc * M + b * 128, (cc + 1) * M
                    nc.tensor.matmul(
                        psD[:, k * stride:k * stride + W],
                        sLd[0:K_DELTA, lo:lo + 128],
                        sR[0:K_DELTA, lo:hi],
                        start=True, stop=True,
                    )

                S_v = psS[:].rearrange("p (c w) -> p c w", c=n_cl)[:, :, 0:W]
                D_v = psD[:].rearrange("p (c w) -> p c w", c=n_cl)[:, :, 0:W]

                # r = 1/(2 th sqrt(sq_s+eps))  [ACT]
                r = work_pool.tile([128, n_cl * W], bf16, tag="r", name="r")
                r_v = r[:].rearrange("p (c w) -> p c w", c=n_cl)
                nc.scalar.activation(
                    r_v, S_v, Act.Abs_reciprocal_sqrt, scale=RSCALE
                )

                # w = delta * r  [DVE, PSUM fp32 x bf16 -> bf16]
                w = work_pool.tile([128, n_cl * W], bf16, tag="w", name="w")
                w_v = w[:].rearrange("p (c w) -> p c w", c=n_cl)
                nc.vector.tensor_tensor(w_v, D_v, r_v, Alu.mult)

                # wc = clamp(w, [-1,1]); DVE 4x, right behind the mult
                wc = work_pool.tile([128, n_cl * W], bf16, tag="wc", name="wc")
                nc.vector.tensor_scalar(
                    wc[:], w[:], 1.0, -1.0, Alu.min, Alu.max
                )
                return wc

            def emit_tail(u, wc):
                """square+accum (ACT) and diag re-accum (DVE) for unit u."""
                n_cl, idx, b = UNITS[u]
                W = M - b * 128
                # acc[u] = sum(wc^2); scr = wc^2 feeds the diag re-sum;
                # host computes full = 2*acc - acc_diag.
                scr = work_pool.tile([128, n_cl * W], bf16, tag="scr", name="scr")
                if b == 3:  # last tail on DVE so ACT's span ends earlier
                    nc.vector.scalar_tensor_tensor(
                        scr[:], wc[:], 1.0, wc[:], Alu.mult, Alu.mult,
                        accum_out=acc[:, u:u + 1],
                    )
                else:
                    nc.scalar.activation(
                        scr[:], wc[:], Act.Square, accum_out=acc[:, u:u + 1],
                    )
                if b < 3:  # b3 strips ARE diag blocks; host reuses acc[u]
                    scr_v = scr[:].rearrange("p (c w) -> p c w", c=n_cl)
                    scrD = work_pool.tile(
                        [128, n_cl * 128], bf16, tag="scrD", name="scrD"
                    )
                    scrD_v = scrD[:].rearrange("p (c w) -> p c w", c=n_cl)
                    nc.vector.tensor_scalar(
                        scrD_v, scr_v[:, :, 0:128], 0.0, None, Alu.add, Alu.add,
                        accum_out=acc[:, N_UNITS + u:N_UNITS + u + 1],
                    )

            loop_cm = tc.For_i(0, loop_n, 1) if loop_n else contextlib.nullcontext()
            with loop_cm:
              # software-pipelined emission with a 1-unit lag: unit u+1's
              # AbsRsqrt sits ahead of unit u's Square in the ACT FIFO, so
              # the sigma path is never blocked behind the delta chain.
              prev = None
              for u in range(N_UNITS):
                wc = emit_head(u)
                if prev is not None:
                    emit_tail(u - 1, prev)
                prev = wc
              emit_tail(N_UNITS - 1, prev)

            # DMA the per-unit accumulators out; host does the final sum.
            # Issued from the ACT queue, which is idle by then.
            nc.scalar.dma_start(out=d_out[:], in_=acc[:])

    nc.compile()
    return nc


def _get_compiled(loop_n=0):
    key = loop_n
    if key not in _COMPILED:
        _COMPILED[key] = _build_bass(loop_n=loop_n)
    return _COMPILED[key]


def _make_in_maps(pc, tg):
    in_maps = []
    for c in range(N_CORES):
        sl = slice(c * PTS_PER_CORE, (c + 1) * PTS_PER_CORE)
        R, Ld, Ls = _build_operands(pc[sl], tg[sl])
        in_maps.append({"R": R, "Ld": Ld, "Ls": Ls})
    return in_maps


def kernel(flow, pc1, labels, num_clusters):
    from concourse.bass_utils import run_bass_kernel_spmd

    pc = np.ascontiguousarray(np.asarray(pc1, dtype=np.float32)[0])    # [N,3]
    fl = np.ascontiguousarray(np.asarray(flow, dtype=np.float32)[0])   # [N,3]
    tg = (pc + fl).astype(np.float32)

    in_maps = _make_in_maps(pc, tg)
    nc = _get_compiled()
    res = run_bass_kernel_spmd(nc, in_maps, core_ids=list(range(N_CORES)))
    total = 0.0
    for r in res.results:
        total += 2.0 * float(r["out"][:, :N_UNITS].sum(dtype=np.float64))
        total -= float(r["out"][:, N_UNITS:].sum(dtype=np.float64))
        for u in B3_UNITS:  # b3 strips are pure diag blocks
            total -= float(r["out"][:, u].sum(dtype=np.float64))
    loss = total / (M * M * NUM_CLUSTERS)
    return np.float32(loss)


def _numpy_check():
    """Validate the math (not the PE rounding) against the reference formula."""
    rng = np.random.default_rng(0)
    P = (rng.standard_normal((1024, 3)) * 20.0).astype(np.float32)
    F = (rng.standard_normal((1024, 3)) * 0.1).astype(np.float32)
    T = P + F
    # reference per 512-cluster
    tot_ref = 0.0
    tot_new = 0.0
    for c in range(2):
        p = P[c * 512:(c + 1) * 512].astype(np.float64)
        t = T[c * 512:(c + 1) * 512].astype(np.float64)
        ds = np.sqrt(((p[:, None] - p[None]) ** 2).sum(-1))
        dt = np.sqrt(((t[:, None] - t[None]) ** 2).sum(-1))
        tot_ref += np.minimum((ds - dt) ** 2, TH2).sum() / TH2
        sqs = ((p[:, None] - p[None]) ** 2).sum(-1)
        sqt = ((t[:, None] - t[None]) ** 2).sum(-1)
        delta = sqs - sqt
        r = 1.0 / np.sqrt(4 * TH2 * (sqs + EPS))
        w = np.clip(delta * r, -1, 1)
        tot_new += (w ** 2).sum()
    print("numpy rel err:", abs(tot_new - tot_ref) / tot_ref)


if __name__ == "__main__":
    _numpy_check()


# revision 48
# speedup vs baseline: 1.1815x; 1.0113x over previous
"""Trainium2 Bass kernel for the clustered spatial-consistency (SC2-PCR) loss.

Problem: 64 contiguous clusters of 512 points each (N=32768, 3-D). Per
cluster compute the 512x512 pairwise-distance matrices of src (pc1) and
tgt (pc1+flow); loss = mean(min((d_s-d_t)^2, th^2)/th^2) over all pairs
and clusters. Sharded 8 clusters per core across 8 NeuronCores.

Math (division form instead of two sqrts):
    v = d_s - d_t = (sq_s - sq_t) / (d_s + d_t)
    (d_s + d_t)^2 ~= 4*(sq_s + eps)   [self-limiting error: the approx
        error is O(v/d) relative, and only |v|<=th pairs matter, where
        v/d <= th/d_min ~ 0.4%]
    w = v/th = delta * r,  delta = sq_s - sq_t  (PE, K=42 matmul)
    r = AbsRsqrt(4*th^2 * (sq_s + eps))         (ACT, one table, exact
        to 4e-5; Rsqrt/Reciprocal are banned but Abs_reciprocal_sqrt
        is accurate)
    sq_s + eps comes from a second cheap matmul  (PE, K=13)
    loss elem = min(w^2, 1)

Per 128-row block only columns >= block start are computed (symmetry):
full sum = 2*strip_sums - diag_block_sums (b=3 strips ARE diag blocks,
so the host reuses their strip sums as diag sums).

Engines per unit (n_cl clusters x one row block; separate sigma/delta
PSUM pools so sigma tiles free right after AbsRsqrt and the PE streams):
    PE:   2*n_cl matmuls (bf16: K=13 sigma, K=42 delta), W=512-128b cols
    ACT:  r = AbsRsqrt(S*scale) PSUM->bf16; Square+accum_out (strip sums)
    DVE:  w = delta*r (PSUM fp32 x bf16, 1x); clamp (bf16 4x);
          diag re-accumulate from scr = wc^2 (tensor_scalar add+accum)
Measured notes: DVE accum_out ops run at ~1x; non-accum bf16
tensor_scalar hits 4x; GpSimd tensor ops are ~0.42-0.73 efficiency and
PSUM-blind, so it only idles; Reciprocal/Rsqrt are banned but
Abs_reciprocal_sqrt measures 4e-5 max rel err on HW.

Operand layout (host-packed bf16, shared moving operand R):
  rows 0-8:  s-products (h,m,h)/coord; Ld=-2(h,h,m), Ls=-2(h,h,m)
  rows 9-10: R=1;  Ls=split2(ns+eps/2), Ld=split2(ns-nt)
  rows 11-12: R=split2(ns+eps/2) j-side; Ls=1, Ld=0
  -> sigma matmul is rows 0:13 (contiguous K=13)
  rows 13-14: R=split2(ns-nt) j-side; Ld=1
  rows 15-23: s-products (m,l,h)/coord; Ld=-2(m,h,l)
  rows 24-41: t-products 6/coord; Ld=+2
  -> delta matmul is rows 0:42
"""

import numpy as np
import ml_dtypes

N_POINTS = 32768
NUM_CLUSTERS = 64
M = N_POINTS // NUM_CLUSTERS          # 512 points per cluster
N_CORES = 8
CLUSTERS_PER_CORE = NUM_CLUSTERS // N_CORES   # 8
PTS_PER_CORE = CLUSTERS_PER_CORE * M  # 4096
D_THRE = 0.03
TH2 = D_THRE * D_THRE
EPS = 0.25
K_DELTA = 42
K_SIGMA = 13
N_BLOCKS = M // 128                   # 4 row blocks per cluster

# units: (n_clusters, cluster-group index, row-block). Each unit fills one
# [128,1024] sigma PSUM tile and one [128,1024] delta tile (2 banks each).
UNITS = (
    [(2, 0, 0), (2, 1, 0), (2, 0, 1), (2, 1, 1), (4, 0, 2)]
    + [(2, 2, 0), (2, 3, 0), (2, 2, 1), (2, 3, 1), (4, 1, 2)]
    + [(8, 0, 3)]
)
N_UNITS = len(UNITS)
B3_UNITS = [u for u, (_, _, b) in enumerate(UNITS) if b == 3]

_COMPILED = {}


def _split3(x):
    x = x.astype(np.float32)
    h = x.astype(ml_dtypes.bfloat16)
    r = x - h.astype(np.float32)
    m = r.astype(ml_dtypes.bfloat16)
    l = (r - m.astype(np.float32)).astype(ml_dtypes.bfloat16)
    return h, m, l


def _split2(x):
    x = x.astype(np.float32)
    h = x.astype(ml_dtypes.bfloat16)
    l = (x - h.astype(np.float32)).astype(ml_dtypes.bfloat16)
    return h, l


def _build_operands(P, T):
    """P, T: [4096, 3] fp32 src/tgt points -> R[42,n], Ld[42,n], Ls[13,n]."""
    bf16 = ml_dtypes.bfloat16
    n = P.shape[0]
    R = np.zeros((K_DELTA, n), dtype=bf16)
    Ld = np.zeros((K_DELTA, n), dtype=bf16)
    Ls = np.zeros((K_SIGMA, n), dtype=bf16)
    hs, ms, ls = [], [], []
    ht, mt, lt = [], [], []
    for c in range(3):
        a, b, d = _split3(P[:, c])
        hs.append(a); ms.append(b); ls.append(d)
        a, b, d = _split3(T[:, c])
        ht.append(a); mt.append(b); lt.append(d)

    def neg2(x):
        return (-2.0 * x.astype(np.float32)).astype(bf16)

    def pos2(x):
        return (2.0 * x.astype(np.float32)).astype(bf16)

    # rows 0-8: s products hh, hm, mh
    for c in range(3):
        R[3 * c + 0] = hs[c]; Ld[3 * c + 0] = neg2(hs[c])
        R[3 * c + 1] = ms[c]; Ld[3 * c + 1] = neg2(hs[c])
        R[3 * c + 2] = hs[c]; Ld[3 * c + 2] = neg2(ms[c])
    Ls[0:9] = Ld[0:9]

    ns = np.einsum("nc,nc->n", P.astype(np.float64), P.astype(np.float64))
    nt = np.einsum("nc,nc->n", T.astype(np.float64), T.astype(np.float64))
    sn_h, sn_l = _split2((ns + EPS / 2).astype(np.float32))
    dn_h, dn_l = _split2((ns - nt).astype(np.float32))
    one = np.ones(n, dtype=bf16)
    # rows 9-10: i-side norms (R=1)
    R[9] = one; Ls[9] = sn_h; Ld[9] = dn_h
    R[10] = one; Ls[10] = sn_l; Ld[10] = dn_l
    # rows 11-12: sigma j-side norms
    R[11] = sn_h; Ls[11] = one
    R[12] = sn_l; Ls[12] = one
    # rows 13-14: delta j-side norms
    R[13] = dn_h; Ld[13] = one
    R[14] = dn_l; Ld[14] = one
    # rows 15-23: s products mm, hl, lh
    for c in range(3):
        R[15 + 3 * c + 0] = ms[c]; Ld[15 + 3 * c + 0] = neg2(ms[c])
        R[15 + 3 * c + 1] = ls[c]; Ld[15 + 3 * c + 1] = neg2(hs[c])
        R[15 + 3 * c + 2] = hs[c]; Ld[15 + 3 * c + 2] = neg2(ls[c])
    # rows 24-41: t products hh, hm, mh, mm, hl, lh (+2)
    for c in range(3):
        base = 24 + 6 * c
        R[base + 0] = ht[c]; Ld[base + 0] = pos2(ht[c])
        R[base + 1] = mt[c]; Ld[base + 1] = pos2(ht[c])
        R[base + 2] = ht[c]; Ld[base + 2] = pos2(mt[c])
        R[base + 3] = mt[c]; Ld[base + 3] = pos2(mt[c])
        R[base + 4] = lt[c]; Ld[base + 4] = pos2(ht[c])
        R[base + 5] = ht[c]; Ld[base + 5] = pos2(lt[c])
    return R, Ld, Ls


def _build_bass(loop_n=0):
    import contextlib
    import concourse.bacc as bacc
    import concourse.mybir as mybir
    import concourse.tile as tile

    f32 = mybir.dt.float32
    bf16 = mybir.dt.bfloat16
    Alu = mybir.AluOpType
    Act = mybir.ActivationFunctionType

    nc = bacc.Bacc("TRN2", target_bir_lowering=False, debug=False)

    d_R = nc.dram_tensor("R", [K_DELTA, PTS_PER_CORE], bf16, kind="ExternalInput")
    d_Ld = nc.dram_tensor("Ld", [K_DELTA, PTS_PER_CORE], bf16, kind="ExternalInput")
    d_Ls = nc.dram_tensor("Ls", [K_SIGMA, PTS_PER_CORE], bf16, kind="ExternalInput")
    d_out = nc.dram_tensor("out", [128, 2 * N_UNITS], f32, kind="ExternalOutput")

    RSCALE = 4.0 * TH2  # r = 1/sqrt(RSCALE*(sq_s+eps)) = 1/(2 th sqrt(sq+eps))

    with tile.TileContext(nc) as tc:
        with (
            tc.tile_pool(name="ops", bufs=1) as ops_pool,
            tc.tile_pool(name="psA", bufs=2, space="PSUM") as psA_pool,
            tc.tile_pool(name="psB", bufs=2, space="PSUM") as psB_pool,
            tc.tile_pool(name="work", bufs=6) as work_pool,
            tc.tile_pool(name="accp", bufs=1) as acc_pool,
        ):
            sR = ops_pool.tile([K_DELTA, PTS_PER_CORE], bf16, tag="sR")
            sLs = ops_pool.tile([K_SIGMA, PTS_PER_CORE], bf16, tag="sLs")
            sLd = ops_pool.tile([K_DELTA, PTS_PER_CORE], bf16, tag="sLd")
            # Ls (13 rows) first, then R: primes the sigma->AbsRsqrt path
            # ~2us before Ld (delta operand) lands
            nc.sync.dma_start(out=sLs[:], in_=d_Ls[:])
            nc.sync.dma_start(out=sR[:], in_=d_R[:])
            nc.sync.dma_start(out=sLd[:], in_=d_Ld[:])

            # acc[:, u] = strip sums; acc[:, N_UNITS+u] = diag sums
            acc = acc_pool.tile([128, 2 * N_UNITS], f32, tag="acc")
            nc.vector.memset(acc[:], 0.0)

            # force the ACT table load now, during the input-DMA wait
            dummy = acc_pool.tile([128, 1], f32, tag="dummy")
            nc.vector.memset(dummy[:], 1.0)
            nc.scalar.activation(dummy[:], dummy[:], Act.Abs_reciprocal_sqrt)

            def emit_head(u):
                """matmuls + AbsRsqrt + mult + clamp for unit u; returns wc."""
                n_cl, idx, b = UNITS[u]
                W = M - b * 128
                stride = 1024 // n_cl   # psum offset per cluster (>= W)
                clusters = [idx * n_cl + k for k in range(n_cl)]

                # separate sigma/delta PSUM tiles: sigma frees right after
                # AbsRsqrt, so the PE can run ahead into the next unit
                psS = psA_pool.tile([128, 1024], f32, tag="psS", name="psS")
                psD = psB_pool.tile([128, 1024], f32, tag="psD", name="psD")
                for k, cc in enumerate(clusters):
                    lo, hi = cc * M + b * 128, (cc + 1) * M
                    nc.tensor.matmul(
                        psS[:, k * stride:k * stride + W],
                        sLs[0:K_SIGMA, lo:lo + 128],
                        sR[0:K_SIGMA, lo:hi],
                        start=True, stop=True,
                    )
                for k, cc in enumerate(clusters):
                    lo, hi = cc * M + b * 128, (cc + 1) * M
                    nc.tensor.matmul(
                        psD[:, k * stride:k * stride + W],
                        sLd[0:K_DELTA, lo:lo + 128],
                        sR[0:K_DELTA, lo:hi],
                        start=True, stop=True,
                    )

                S_v = psS[:].rearrange("p (c w) -> p c w", c=n_cl)[:, :, 0:W]
                D_v = psD[:].rearrange("p (c w) -> p c w", c=n_cl)[:, :, 0:W]

                # r = 1/(2 th sqrt(sq_s+eps))  [ACT]
                r = work_pool.tile([128, n_cl * W], bf16, tag="r", name="r")
                r_v = r[:].rearrange("p (c w) -> p c w", c=n_cl)
                nc.scalar.activation(
                    r_v, S_v, Act.Abs_reciprocal_sqrt, scale=RSCALE
                )

                # w = delta * r  [DVE, PSUM fp32 x bf16 -> bf16]
                w = work_pool.tile([128, n_cl * W], bf16, tag="w", name="w")
                w_v = w[:].rearrange("p (c w) -> p c w", c=n_cl)
                nc.vector.tensor_tensor(w_v, D_v, r_v, Alu.mult)

                # wc = clamp(w, [-1,1]); DVE 4x, right behind the mult
                wc = work_pool.tile([128, n_cl * W], bf16, tag="wc", name="wc")
                nc.vector.tensor_scalar(
                    wc[:], w[:], 1.0, -1.0, Alu.min, Alu.max
                )
                return wc

            def emit_tail(u, wc):
                """square+accum (ACT) and diag re-accum (DVE) for unit u."""
                n_cl, idx, b = UNITS[u]
                W = M - b * 128
                # acc[u] = sum(wc^2); scr = wc^2 feeds the diag re-sum;
                # host computes full = 2*acc - acc_diag.
                scr = work_pool.tile([128, n_cl * W], bf16, tag="scr", name="scr")
                if b == 3:  # last tail on DVE so ACT's span ends earlier
                    nc.vector.scalar_tensor_tensor(
                        scr[:], wc[:], 1.0, wc[:], Alu.mult, Alu.mult,
                        accum_out=acc[:, u:u + 1],
                    )
                else:
                    nc.scalar.activation(
                        scr[:], wc[:], Act.Square, accum_out=acc[:, u:u + 1],
                    )
                if b < 3:  # b3 strips ARE diag blocks; host reuses acc[u]
                    scr_v = scr[:].rearrange("p (c w) -> p c w", c=n_cl)
                    scrD = work_pool.tile(
                        [128, n_cl * 128], bf16, tag="scrD", name="scrD"
                    )
                    scrD_v = scrD[:].rearrange("p (c w) -> p c w", c=n_cl)
                    nc.vector.tensor_scalar(
                        scrD_v, scr_v[:, :, 0:128], 0.0, None, Alu.add, Alu.add,
                        accum_out=acc[:, N_UNITS + u:N_UNITS + u + 1],
                    )

            loop_cm = tc.For_i(0, loop_n, 1) if loop_n else contextlib.nullcontext()
            with loop_cm:
              # software-pipelined emission with a 2-unit lag: units u+1
              # and u+2's AbsRsqrt sit ahead of unit u's Square in the ACT
              # FIFO, so the sigma path never blocks behind the delta chain.
              pend = []
              for u in range(N_UNITS):
                wc = emit_head(u)
                pend.append((u, wc))
                if len(pend) > 2:
                    pu, pwc = pend.pop(0)
                    emit_tail(pu, pwc)
              for pu, pwc in pend:
                emit_tail(pu, pwc)

            # DMA the per-unit accumulators out; host does the final sum.
            # Issued from the ACT queue, which is idle by then.
            nc.scalar.dma_start(out=d_out[:], in_=acc[:])

    nc.compile()
    return nc


def _get_compiled(loop_n=0):
    key = loop_n
    if key not in _COMPILED:
        _COMPILED[key] = _build_bass(loop_n=loop_n)
    return _COMPILED[key]


def _make_in_maps(pc, tg):
    in_maps = []
    for c in range(N_CORES):
        sl = slice(c * PTS_PER_CORE, (c + 1) * PTS_PER_CORE)
        R, Ld, Ls = _build_operands(pc[sl], tg[sl])
        in_maps.append({"R": R, "Ld": Ld, "Ls": Ls})
    return in_maps


def kernel(flow, pc1, labels, num_clusters):
    from concourse.bass_utils import run_bass_kernel_spmd

    pc = np.ascontiguousarray(np.asarray(pc1, dtype=np.float32)[0])    # [N,3]
    fl = np.ascontiguousarray(np.asarray(flow, dtype=np.float32)[0])   # [N,3]
    tg = (pc + fl).astype(np.float32)

    in_maps = _make_in_maps(pc, tg)
    nc = _get_compiled()
    res = run_bass_kernel_spmd(nc, in_maps, core_ids=list(range(N_CORES)))
    total = 0.0
    for r in res.results:
        total += 2.0 * float(r["out"][:, :N_UNITS].sum(dtype=np.float64))
        total -= float(r["out"][:, N_UNITS:].sum(dtype=np.float64))
        for u in B3_UNITS:  # b3 strips are pure diag blocks
            total -= float(r["out"][:, u].sum(dtype=np.float64))
    loss = total / (M * M * NUM_CLUSTERS)
    return np.float32(loss)


def _numpy_check():
    """Validate the math (not the PE rounding) against the reference formula."""
    rng = np.random.default_rng(0)
    P = (rng.standard_normal((1024, 3)) * 20.0).astype(np.float32)
    F = (rng.standard_normal((1024, 3)) * 0.1).astype(np.float32)
    T = P + F
    # reference per 512-cluster
    tot_ref = 0.0
    tot_new = 0.0
    for c in range(2):
        p = P[c * 512:(c + 1) * 512].astype(np.float64)
        t = T[c * 512:(c + 1) * 512].astype(np.float64)
        ds = np.sqrt(((p[:, None] - p[None]) ** 2).sum(-1))
        dt = np.sqrt(((t[:, None] - t[None]) ** 2).sum(-1))
        tot_ref += np.minimum((ds - dt) ** 2, TH2).sum() / TH2
        sqs = ((p[:, None] - p[None]) ** 2).sum(-1)
        sqt = ((t[:, None] - t[None]) ** 2).sum(-1)
        delta = sqs - sqt
        r = 1.0 / np.sqrt(4 * TH2 * (sqs + EPS))
        w = np.clip(delta * r, -1, 1)
        tot_new += (w ** 2).sum()
    print("numpy rel err:", abs(tot_new - tot_ref) / tot_ref)


if __name__ == "__main__":
    _numpy_check()


# revision 49
# speedup vs baseline: 1.2028x; 1.0180x over previous
"""Trainium2 Bass kernel for the clustered spatial-consistency (SC2-PCR) loss.

Problem: 64 contiguous clusters of 512 points each (N=32768, 3-D). Per
cluster compute the 512x512 pairwise-distance matrices of src (pc1) and
tgt (pc1+flow); loss = mean(min((d_s-d_t)^2, th^2)/th^2) over all pairs
and clusters. Sharded 8 clusters per core across 8 NeuronCores.

Math (division form instead of two sqrts):
    v = d_s - d_t = (sq_s - sq_t) / (d_s + d_t)
    (d_s + d_t)^2 ~= 4*(sq_s + eps)   [self-limiting error: the approx
        error is O(v/d) relative, and only |v|<=th pairs matter, where
        v/d <= th/d_min ~ 0.4%]
    w = v/th = delta * r,  delta = sq_s - sq_t  (PE, K=42 matmul)
    r = AbsRsqrt(4*th^2 * (sq_s + eps))         (ACT, one table, exact
        to 4e-5; Rsqrt/Reciprocal are banned but Abs_reciprocal_sqrt
        is accurate)
    sq_s + eps comes from a second cheap matmul  (PE, K=13)
    loss elem = min(w^2, 1)

Per 128-row block only columns >= block start are computed (symmetry):
full sum = 2*strip_sums - diag_block_sums (b=3 strips ARE diag blocks,
so the host reuses their strip sums as diag sums).

Engines per unit (n_cl clusters x one row block; separate sigma/delta
PSUM pools so sigma tiles free right after AbsRsqrt and the PE streams):
    PE:   2*n_cl matmuls (bf16: K=13 sigma, K=42 delta), W=512-128b cols
    ACT:  r = AbsRsqrt(S*scale) PSUM->bf16; Square+accum_out (strip sums)
    DVE:  w = delta*r (PSUM fp32 x bf16, 1x); clamp (bf16 4x);
          diag re-accumulate from scr = wc^2 (tensor_scalar add+accum)
Measured notes: DVE accum_out ops run at ~1x; non-accum bf16
tensor_scalar hits 4x; GpSimd tensor ops are ~0.42-0.73 efficiency and
PSUM-blind, so it only idles; Reciprocal/Rsqrt are banned but
Abs_reciprocal_sqrt measures 4e-5 max rel err on HW.

Operand layout (host-packed bf16, shared moving operand R):
  rows 0-8:  s-products (h,m,h)/coord; Ld=-2(h,h,m), Ls=-2(h,h,m)
  rows 9-10: R=1;  Ls=split2(ns+eps/2), Ld=split2(ns-nt)
  rows 11-12: R=split2(ns+eps/2) j-side; Ls=1, Ld=0
  -> sigma matmul is rows 0:13 (contiguous K=13)
  rows 13-14: R=split2(ns-nt) j-side; Ld=1
  rows 15-23: s-products (m,l,h)/coord; Ld=-2(m,h,l)
  rows 24-41: t-products 6/coord; Ld=+2
  -> delta matmul is rows 0:42
"""

import numpy as np
import ml_dtypes

N_POINTS = 32768
NUM_CLUSTERS = 64
M = N_POINTS // NUM_CLUSTERS          # 512 points per cluster
N_CORES = 8
CLUSTERS_PER_CORE = NUM_CLUSTERS // N_CORES   # 8
PTS_PER_CORE = CLUSTERS_PER_CORE * M  # 4096
D_THRE = 0.03
TH2 = D_THRE * D_THRE
EPS = 0.25
K_DELTA = 42
K_SIGMA = 13
N_BLOCKS = M // 128                   # 4 row blocks per cluster

# units: (n_clusters, cluster-group index, row-block). Each unit fills one
# [128,1024] sigma PSUM tile and one [128,1024] delta tile (2 banks each).
UNITS = (
    [(2, 0, 0), (2, 1, 0), (2, 0, 1), (2, 1, 1), (4, 0, 2)]
    + [(2, 2, 0), (2, 3, 0), (2, 2, 1), (2, 3, 1), (4, 1, 2)]
    + [(8, 0, 3)]
)
N_UNITS = len(UNITS)
B3_UNITS = [u for u, (_, _, b) in enumerate(UNITS) if b == 3]

_COMPILED = {}


def _split3(x):
    x = x.astype(np.float32)
    h = x.astype(ml_dtypes.bfloat16)
    r = x - h.astype(np.float32)
    m = r.astype(ml_dtypes.bfloat16)
    l = (r - m.astype(np.float32)).astype(ml_dtypes.bfloat16)
    return h, m, l


def _split2(x):
    x = x.astype(np.float32)
    h = x.astype(ml_dtypes.bfloat16)
    l = (x - h.astype(np.float32)).astype(ml_dtypes.bfloat16)
    return h, l


def _build_operands(P, T):
    """P, T: [4096, 3] fp32 src/tgt points -> R[42,n], Ld[42,n], Ls[13,n]."""
    bf16 = ml_dtypes.bfloat16
    n = P.shape[0]
    R = np.zeros((K_DELTA, n), dtype=bf16)
    Ld = np.zeros((K_DELTA, n), dtype=bf16)
    Ls = np.zeros((K_SIGMA, n), dtype=bf16)
    hs, ms, ls = [], [], []
    ht, mt, lt = [], [], []
    for c in range(3):
        a, b, d = _split3(P[:, c])
        hs.append(a); ms.append(b); ls.append(d)
        a, b, d = _split3(T[:, c])
        ht.append(a); mt.append(b); lt.append(d)

    def neg2(x):
        return (-2.0 * x.astype(np.float32)).astype(bf16)

    def pos2(x):
        return (2.0 * x.astype(np.float32)).astype(bf16)

    # rows 0-8: s products hh, hm, mh
    for c in range(3):
        R[3 * c + 0] = hs[c]; Ld[3 * c + 0] = neg2(hs[c])
        R[3 * c + 1] = ms[c]; Ld[3 * c + 1] = neg2(hs[c])
        R[3 * c + 2] = hs[c]; Ld[3 * c + 2] = neg2(ms[c])
    Ls[0:9] = Ld[0:9]

    ns = np.einsum("nc,nc->n", P.astype(np.float64), P.astype(np.float64))
    nt = np.einsum("nc,nc->n", T.astype(np.float64), T.astype(np.float64))
    sn_h, sn_l = _split2((ns + EPS / 2).astype(np.float32))
    dn_h, dn_l = _split2((ns - nt).astype(np.float32))
    one = np.ones(n, dtype=bf16)
    # rows 9-10: i-side norms (R=1)
    R[9] = one; Ls[9] = sn_h; Ld[9] = dn_h
    R[10] = one; Ls[10] = sn_l; Ld[10] = dn_l
    # rows 11-12: sigma j-side norms
    R[11] = sn_h; Ls[11] = one
    R[12] = sn_l; Ls[12] = one
    # rows 13-14: delta j-side norms
    R[13] = dn_h; Ld[13] = one
    R[14] = dn_l; Ld[14] = one
    # rows 15-23: s products mm, hl, lh
    for c in range(3):
        R[15 + 3 * c + 0] = ms[c]; Ld[15 + 3 * c + 0] = neg2(ms[c])
        R[15 + 3 * c + 1] = ls[c]; Ld[15 + 3 * c + 1] = neg2(hs[c])
        R[15 + 3 * c + 2] = hs[c]; Ld[15 + 3 * c + 2] = neg2(ls[c])
    # rows 24-41: t products hh, hm, mh, mm, hl, lh (+2)
    for c in range(3):
        base = 24 + 6 * c
        R[base + 0] = ht[c]; Ld[base + 0] = pos2(ht[c])
        R[base + 1] = mt[c]; Ld[base + 1] = pos2(ht[c])
        R[base + 2] = ht[c]; Ld[base + 2] = pos2(mt[c])
        R[base + 3] = mt[c]; Ld[base + 3] = pos2(mt[c])
        R[base + 4] = lt[c]; Ld[base + 4] = pos2(ht[c])
        R[base + 5] = ht[c]; Ld[base + 5] = pos2(lt[c])
    return R, Ld, Ls


def _build_bass(loop_n=0):
    import contextlib
    import concourse.bacc as bacc
    import concourse.mybir as mybir
    import concourse.tile as tile

    f32 = mybir.dt.float32
    bf16 = mybir.dt.bfloat16
    Alu = mybir.AluOpType
    Act = mybir.ActivationFunctionType

    nc = bacc.Bacc("TRN2", target_bir_lowering=False, debug=False)

    d_R = nc.dram_tensor("R", [K_DELTA, PTS_PER_CORE], bf16, kind="ExternalInput")
    d_Ld = nc.dram_tensor("Ld", [K_DELTA, PTS_PER_CORE], bf16, kind="ExternalInput")
    d_Ls = nc.dram_tensor("Ls", [K_SIGMA, PTS_PER_CORE], bf16, kind="ExternalInput")
    d_out = nc.dram_tensor("out", [128, 2 * N_UNITS], f32, kind="ExternalOutput")

    RSCALE = 4.0 * TH2  # r = 1/sqrt(RSCALE*(sq_s+eps)) = 1/(2 th sqrt(sq+eps))

    with tile.TileContext(nc) as tc:
        with (
            tc.tile_pool(name="ops", bufs=1) as ops_pool,
            tc.tile_pool(name="psA", bufs=2, space="PSUM") as psA_pool,
            tc.tile_pool(name="psB", bufs=2, space="PSUM") as psB_pool,
            tc.tile_pool(name="work", bufs=6) as work_pool,
            tc.tile_pool(name="accp", bufs=1) as acc_pool,
        ):
            sR = ops_pool.tile([K_DELTA, PTS_PER_CORE], bf16, tag="sR")
            sLs = ops_pool.tile([K_SIGMA, PTS_PER_CORE], bf16, tag="sLs")
            sLd = ops_pool.tile([K_DELTA, PTS_PER_CORE], bf16, tag="sLd")
            # Ls (13 rows) first, then R: primes the sigma->AbsRsqrt path
            # ~2us before Ld (delta operand) lands
            nc.sync.dma_start(out=sLs[:], in_=d_Ls[:])
            nc.sync.dma_start(out=sR[:], in_=d_R[:])
            nc.sync.dma_start(out=sLd[:], in_=d_Ld[:])

            # acc[:, u] = strip sums; acc[:, N_UNITS+u] = diag sums
            acc = acc_pool.tile([128, 2 * N_UNITS], f32, tag="acc")
            nc.vector.memset(acc[:], 0.0)

            # force the ACT table load now, during the input-DMA wait
            dummy = acc_pool.tile([128, 1], f32, tag="dummy")
            nc.vector.memset(dummy[:], 1.0)
            nc.scalar.activation(dummy[:], dummy[:], Act.Abs_reciprocal_sqrt)

            def emit_head(u):
                """matmuls + AbsRsqrt + mult + clamp for unit u; returns wc."""
                n_cl, idx, b = UNITS[u]
                W = M - b * 128
                stride = 1024 // n_cl   # psum offset per cluster (>= W)
                clusters = [idx * n_cl + k for k in range(n_cl)]

                # separate sigma/delta PSUM tiles: sigma frees right after
                # AbsRsqrt, so the PE can run ahead into the next unit
                psS = psA_pool.tile([128, 1024], f32, tag="psS", name="psS")
                psD = psB_pool.tile([128, 1024], f32, tag="psD", name="psD")
                for k, cc in enumerate(clusters):
                    lo, hi = cc * M + b * 128, (cc + 1) * M
                    nc.tensor.matmul(
                        psS[:, k * stride:k * stride + W],
                        sLs[0:K_SIGMA, lo:lo + 128],
                        sR[0:K_SIGMA, lo:hi],
                        start=True, stop=True,
                    )
                for k, cc in enumerate(clusters):
                    lo, hi = cc * M + b * 128, (cc + 1) * M
                    nc.tensor.matmul(
                        psD[:, k * stride:k * stride + W],
                        sLd[0:K_DELTA, lo:lo + 128],
                        sR[0:K_DELTA, lo:hi],
                        start=True, stop=True,
                    )

                S_v = psS[:].rearrange("p (c w) -> p c w", c=n_cl)[:, :, 0:W]
                D_v = psD[:].rearrange("p (c w) -> p c w", c=n_cl)[:, :, 0:W]

                # r = 1/(2 th sqrt(sq_s+eps))  [ACT]
                r = work_pool.tile([128, n_cl * W], bf16, tag="r", name="r")
                r_v = r[:].rearrange("p (c w) -> p c w", c=n_cl)
                nc.scalar.activation(
                    r_v, S_v, Act.Abs_reciprocal_sqrt, scale=RSCALE
                )

                # w = delta * r  [DVE, PSUM fp32 x bf16 -> bf16]
                w = work_pool.tile([128, n_cl * W], bf16, tag="w", name="w")
                w_v = w[:].rearrange("p (c w) -> p c w", c=n_cl)
                nc.vector.tensor_tensor(w_v, D_v, r_v, Alu.mult)

                # wc = clamp(w, [-1,1]); DVE 4x, right behind the mult
                wc = work_pool.tile([128, n_cl * W], bf16, tag="wc", name="wc")
                nc.vector.tensor_scalar(
                    wc[:], w[:], 1.0, -1.0, Alu.min, Alu.max
                )
                return wc

            def emit_tail(u, wc):
                """square+accum (ACT) and diag re-accum (DVE) for unit u."""
                n_cl, idx, b = UNITS[u]
                W = M - b * 128
                # acc[u] = sum(wc^2); scr = wc^2 feeds the diag re-sum;
                # host computes full = 2*acc - acc_diag.
                scr = work_pool.tile([128, n_cl * W], bf16, tag="scr", name="scr")
                if b == 3:  # last tail on DVE so ACT's span ends earlier
                    nc.vector.scalar_tensor_tensor(
                        scr[:], wc[:], 1.0, wc[:], Alu.mult, Alu.mult,
                        accum_out=acc[:, u:u + 1],
                    )
                else:
                    nc.scalar.activation(
                        scr[:], wc[:], Act.Square, accum_out=acc[:, u:u + 1],
                    )
                if b < 3:  # b3 strips ARE diag blocks; host reuses acc[u]
                    scr_v = scr[:].rearrange("p (c w) -> p c w", c=n_cl)
                    scrD = work_pool.tile(
                        [128, n_cl * 128], bf16, tag="scrD", name="scrD"
                    )
                    scrD_v = scrD[:].rearrange("p (c w) -> p c w", c=n_cl)
                    nc.vector.tensor_scalar(
                        scrD_v, scr_v[:, :, 0:128], 0.0, None, Alu.add, Alu.add,
                        accum_out=acc[:, N_UNITS + u:N_UNITS + u + 1],
                    )

            loop_cm = tc.For_i(0, loop_n, 1) if loop_n else contextlib.nullcontext()
            with loop_cm:
              # software-pipelined emission with a 2-unit lag: units u+1
              # and u+2's AbsRsqrt sit ahead of unit u's Square in the ACT
              # FIFO, so the sigma path never blocks behind the delta chain.
              pend = []
              for u in range(N_UNITS):
                wc = emit_head(u)
                pend.append((u, wc))
                if len(pend) > 3:
                    pu, pwc = pend.pop(0)
                    emit_tail(pu, pwc)
              for pu, pwc in pend:
                emit_tail(pu, pwc)

            # DMA the per-unit accumulators out; host does the final sum.
            # Issued from the ACT queue, which is idle by then.
            nc.scalar.dma_start(out=d_out[:], in_=acc[:])

    nc.compile()
    return nc


def _get_compiled(loop_n=0):
    key = loop_n
    if key not in _COMPILED:
        _COMPILED[key] = _build_bass(loop_n=loop_n)
    return _COMPILED[key]


def _make_in_maps(pc, tg):
    in_maps = []
    for c in range(N_CORES):
        sl = slice(c * PTS_PER_CORE, (c + 1) * PTS_PER_CORE)
        R, Ld, Ls = _build_operands(pc[sl], tg[sl])
        in_maps.append({"R": R, "Ld": Ld, "Ls": Ls})
    return in_maps


def kernel(flow, pc1, labels, num_clusters):
    from concourse.bass_utils import run_bass_kernel_spmd

    pc = np.ascontiguousarray(np.asarray(pc1, dtype=np.float32)[0])    # [N,3]
    fl = np.ascontiguousarray(np.asarray(flow, dtype=np.float32)[0])   # [N,3]
    tg = (pc + fl).astype(np.float32)

    in_maps = _make_in_maps(pc, tg)
    nc = _get_compiled()
    res = run_bass_kernel_spmd(nc, in_maps, core_ids=list(range(N_CORES)))
    total = 0.0
    for r in res.results:
        total += 2.0 * float(r["out"][:, :N_UNITS].sum(dtype=np.float64))
        total -= float(r["out"][:, N_UNITS:].sum(dtype=np.float64))
        for u in B3_UNITS:  # b3 strips are pure diag blocks
            total -= float(r["out"][:, u].sum(dtype=np.float64))
    loss = total / (M * M * NUM_CLUSTERS)
    return np.float32(loss)


def _numpy_check():
    """Validate the math (not the PE rounding) against the reference formula."""
    rng = np.random.default_rng(0)
    P = (rng.standard_normal((1024, 3)) * 20.0).astype(np.float32)
    F = (rng.standard_normal((1024, 3)) * 0.1).astype(np.float32)
    T = P + F
    # reference per 512-cluster
    tot_ref = 0.0
    tot_new = 0.0
    for c in range(2):
        p = P[c * 512:(c + 1) * 512].astype(np.float64)
        t = T[c * 512:(c + 1) * 512].astype(np.float64)
        ds = np.sqrt(((p[:, None] - p[None]) ** 2).sum(-1))
        dt = np.sqrt(((t[:, None] - t[None]) ** 2).sum(-1))
        tot_ref += np.minimum((ds - dt) ** 2, TH2).sum() / TH2
        sqs = ((p[:, None] - p[None]) ** 2).sum(-1)
        sqt = ((t[:, None] - t[None]) ** 2).sum(-1)
        delta = sqs - sqt
        r = 1.0 / np.sqrt(4 * TH2 * (sqs + EPS))
        w = np.clip(delta * r, -1, 1)
        tot_new += (w ** 2).sum()
    print("numpy rel err:", abs(tot_new - tot_ref) / tot_ref)


if __name__ == "__main__":
    _numpy_check()
